# revision 6
# baseline (speedup 1.0000x reference)
"""Trainium2 Bass kernel for Compact Bilinear Pooling (count-sketch + circular conv)
+ signed-sqrt + L2 norm + linear classifier.

Math: y[d] = sum_{c1,c2} x-Gram[c1,c2] * s1[c1] * s2[c2] * [(h1[c1]+h2[c2]) mod D == d]
(equivalent to the reference's per-pixel count-sketch FFT circular convolution summed
over pixels, since the conv factorizes over channel pairs sharing a pixel index).

Device pipeline per core (1 batch element, pure data parallel over B=8):
  1. Gram G'T = X2p^T X1p (bf16 matmuls, fp32 PSUM), c2 rows h2-sorted, c1 rows
     sorted by h1 mod 128.
  2. Scatter-by-h2: chunked one-hot matmuls over sorted rank slices (+rank-1
     "straddler" matmuls for hash values crossing chunk boundaries).
  3. Roll-by-h1 decomposed: fine shift (h1 mod 128) via contiguous row-group
     SBUF->SBUF DMA copies; coarse shift via group-sum matmul (one-hot over
     h1 div 128) + 64 static rotation matmuls (slices of a double identity).
  4. signed-sqrt, L2 normalize (||feat||^2 == sum|y|), classifier matmul.

Hash tables h1/h2 are known at kernel-build time; all hash-dependent structure is
baked into instruction access patterns / constant input tensors.
"""
import numpy as np
import ml_dtypes
from contextlib import ExitStack

import concourse.bacc as bacc
import concourse.mybir as mybir
import concourse.tile as tile
from concourse.bass_utils import run_bass_kernel_spmd

BF16 = ml_dtypes.bfloat16
B, C, D, L = 8, 2048, 8192, 784
NCLS = 200
NCORES = 8
FINE = 128          # fine shift modulus (h1 % FINE)
NQ = D // FINE      # 64 coarse groups (h1 // FINE)
RCHUNK = 64         # h2-sorted ranks per stage-3 chunk
NCHUNK = C // RCHUNK  # 32
BW = 256            # c1 block width for Gram/stage-3 pipelining
NBLK = C // BW      # 8
NT = C // 128       # 16 c1 tiles
DT = mybir.dt

_CACHE = {}


def _host_prep(h1, h2):
    """All hash-derived structure + constant tensors (s-independent parts are
    filled at call time; here we only need h1/h2)."""
    h1 = np.asarray(h1).astype(np.int64)
    h2 = np.asarray(h2).astype(np.int64)
    perm1 = np.argsort(h1 % FINE, kind="stable")
    perm2 = np.argsort(h2, kind="stable")
    h2s = h2[perm2]
    rho = (h1[perm1] % FINE).astype(np.int64)
    qg = (h1[perm1] // FINE).astype(np.int64)

    # value -> chunk of its first rank
    first_rank = {}
    for i in range(C):
        v = int(h2s[i])
        if v not in first_rank:
            first_rank[v] = i
    cofv = {v: fr // RCHUNK for v, fr in first_rank.items()}

    starts = np.zeros(NCHUNK, np.int64)
    ends = np.zeros(NCHUNK, np.int64)
    cur = 0
    for k in range(NCHUNK):
        starts[k] = cur
        vals_k = [v for v, c in cofv.items() if c == k]
        nxt = D if k == NCHUNK - 1 else (max(vals_k) + 1 if vals_k else cur)
        ends[k] = nxt
        cur = nxt
    assert ends[-1] == D
    spans = (ends - starts).astype(np.int64)
    W = int(((spans.max() + 127) // 128) * 128)

    # one-hot columns & straddlers
    sheet_col = np.full(C, -1, np.int64)    # column in OWN chunk frame (main sheet)
    is_straddle = np.zeros(C, bool)
    n_straddle = np.zeros(NCHUNK, np.int64)
    for i in range(C):
        v = int(h2s[i])
        k_own = i // RCHUNK
        k_asg = cofv[v]
        if k_asg == k_own:
            sheet_col[i] = v - starts[k_own]
        else:
            assert k_asg == k_own - 1
            assert v == ends[k_asg] - 1
            is_straddle[i] = True
            n_straddle[k_own] += 1
    # straddlers are a contiguous head of each chunk
    for k in range(NCHUNK):
        ns = int(n_straddle[k])
        assert is_straddle[RCHUNK * k:RCHUNK * k + ns].all()
        assert not is_straddle[RCHUNK * k + ns:RCHUNK * (k + 1)].any()
        assert ns <= RCHUNK

    # fine-roll groups per c1 tile: (p0, n, rho)
    groups = []
    for t in range(NT):
        g = []
        seg = rho[128 * t:128 * (t + 1)]
        p0 = 0
        for p in range(1, 129):
            if p == 128 or seg[p] != seg[p0]:
                g.append((p0, p - p0, int(seg[p0])))
                p0 = p
        groups.append(g)

    return dict(perm1=perm1, perm2=perm2, h2s=h2s, rho=rho, qg=qg,
                starts=starts, ends=ends, spans=spans, W=W,
                sheet_col=sheet_col, is_straddle=is_straddle,
                n_straddle=n_straddle, groups=groups)


def _bake_tensors(meta, s1, s2):
    """Value tensors (s1/s2 folded into one-hots)."""
    perm1, perm2 = meta["perm1"], meta["perm2"]
    W = meta["W"]
    sheet_col, is_straddle = meta["sheet_col"], meta["is_straddle"]
    qg = meta["qg"]

    # main sheet, SBUF layout [128, 16, W]: rank i = 128*m + p -> [p, m, col]
    sheetm = np.zeros((128, NT, W), np.float32)
    svals = np.zeros((128, NT), np.float32)
    for i in range(C):
        p, m = i % 128, i // 128
        if is_straddle[i]:
            svals[p, m] = s2[perm2[i]]
        else:
            sheetm[p, m, sheet_col[i]] = s2[perm2[i]]

    # onehotQ, SBUF layout [128, 16, 64]: j = 128*t + r -> [r, t, q]
    oq = np.zeros((128, NT, NQ), np.float32)
    for j in range(C):
        r, t = j % 128, j // 128
        oq[r, t, qg[j]] = s1[perm1[j]]

    di64 = np.zeros((64, 128), np.float32)
    for m in range(64):
        di64[m, m] = 1.0
        di64[m, m + 64] = 1.0

    return dict(
        sheetm=sheetm.astype(BF16),
        svals=svals.astype(BF16),
        oq=oq.astype(BF16),
        di64=di64.astype(BF16),
        ones64=np.ones((64, 1), np.float32),
        ones164=np.ones((1, 64), np.float32),
    )


def _build(meta):
    nc = bacc.Bacc("TRN2", target_bir_lowering=False, debug=False,
                   num_devices=NCORES)
    W = meta["W"]
    starts, spans = meta["starts"], meta["spans"]
    n_straddle = meta["n_straddle"]
    groups = meta["groups"]

    def din(name, shape, dt):
        return nc.dram_tensor(name, shape, dt, kind="ExternalInput").ap()

    xt1 = din("xt1", [L, C], DT.bfloat16)
    xt2 = din("xt2", [L, C], DT.bfloat16)
    sheetm_d = din("sheetm", [128, NT, W], DT.bfloat16)
    svals_d = din("svals", [128, NT], DT.bfloat16)
    oq_d = din("oq", [128, NT, NQ], DT.bfloat16)
    di64_d = din("di64", [64, 128], DT.bfloat16)
    ones64_d = din("ones64", [64, 1], DT.float32)
    ones164_d = din("ones164", [1, 64], DT.float32)
    wcls_d = din("wcls", [128, 64 * NCLS], DT.float32)
    bcls_d = din("bcls", [1, NCLS], DT.float32)
    feat_out = nc.dram_tensor("feat_out", [64, 128], DT.float32,
                              kind="ExternalOutput").ap()
    logit_out = nc.dram_tensor("logit_out", [1, NCLS], DT.float32,
                               kind="ExternalOutput").ap()

    with tile.TileContext(nc) as tc, ExitStack() as ctx:
        consts = ctx.enter_context(tc.tile_pool(name="consts", bufs=1))
        xtp = ctx.enter_context(tc.tile_pool(name="xtp", bufs=1))
        bigp = ctx.enter_context(tc.tile_pool(name="bigp", bufs=1))
        gtp = ctx.enter_context(tc.tile_pool(name="gtp", bufs=2))
        arawp = ctx.enter_context(tc.tile_pool(name="arawp", bufs=2))
        afp = ctx.enter_context(tc.tile_pool(name="afp", bufs=2))
        qp = ctx.enter_context(tc.tile_pool(name="qp", bufs=1))
        postp = ctx.enter_context(tc.tile_pool(name="postp", bufs=1))
        psg = ctx.enter_context(tc.tile_pool(name="psg", bufs=2, space="PSUM"))
        ps3 = ctx.enter_context(tc.tile_pool(name="ps3", bufs=2, space="PSUM"))
        ps4 = ctx.enter_context(tc.tile_pool(name="ps4", bufs=2, space="PSUM"))
        psz = ctx.enter_context(tc.tile_pool(name="psz", bufs=1, space="PSUM"))

        # ---- constant loads
        sheetm_s = bigp.tile([128, NT, W], DT.bfloat16, tag="sheet")
        nc.sync.dma_start(sheetm_s[:], sheetm_d[:])
        svals_s = consts.tile([128, NT], DT.bfloat16, tag="svals")
        nc.sync.dma_start(svals_s[:], svals_d[:])
        oq_s = consts.tile([128, NT, NQ], DT.bfloat16, tag="oq")
        nc.sync.dma_start(oq_s[:], oq_d[:])
        di64_s = consts.tile([64, 128], DT.bfloat16, tag="di64")
        nc.sync.dma_start(di64_s[:], di64_d[:])
        ones64_s = consts.tile([64, 1], DT.float32, tag="ones64")
        nc.sync.dma_start(ones64_s[:], ones64_d[:])
        ones164_s = consts.tile([1, 64], DT.float32, tag="ones164")
        nc.sync.dma_start(ones164_s[:], ones164_d[:])
        bcls_s = consts.tile([1, NCLS], DT.float32, tag="bcls")
        nc.sync.dma_start(bcls_s[:], bcls_d[:])

        # ---- X^T loads (7 L-partition tiles per side)
        xt1_t, xt2_t = [], []
        for p in range(7):
            rows = 128 if p < 6 else L - 768
            t1 = xtp.tile([rows, C], DT.bfloat16, tag=f"xt1_{p}")
            nc.sync.dma_start(t1[:], xt1[128 * p:128 * p + rows, :])
            xt1_t.append(t1)
            t2 = xtp.tile([rows, C], DT.bfloat16, tag=f"xt2_{p}")
            nc.sync.dma_start(t2[:], xt2[128 * p:128 * p + rows, :])
            xt2_t.append(t2)

        q_s = qp.tile([64, D], DT.float32, tag="q")
        af_tiles = [None] * NT

        def emit_4a(j):
            for n in range(16):
                pt = ps4.tile([64, 512], DT.float32, tag="ps4")
                nc.tensor.matmul(pt[:], oq_s[:, 2 * j, :],
                                 af_tiles[2 * j][:, 512 * n:512 * (n + 1)],
                                 start=True, stop=False)
                nc.tensor.matmul(pt[:], oq_s[:, 2 * j + 1, :],
                                 af_tiles[2 * j + 1][:, 512 * n:512 * (n + 1)],
                                 start=False, stop=True)
                if j == 0:
                    nc.vector.tensor_copy(q_s[:, 512 * n:512 * (n + 1)], pt[:])
                else:
                    nc.vector.tensor_tensor(q_s[:, 512 * n:512 * (n + 1)],
                                            q_s[:, 512 * n:512 * (n + 1)],
                                            pt[:], op=mybir.AluOpType.add)
            af_tiles[2 * j] = None
            af_tiles[2 * j + 1] = None

        for blk in range(NBLK):
            # ---- Gram block: G'T[:, blk*BW : (blk+1)*BW]
            gt = gtp.tile([128, NT, BW], DT.bfloat16, tag="gt")
            for m in range(NT):
                pg = psg.tile([128, BW], DT.float32, tag="psg")
                for p in range(7):
                    nc.tensor.matmul(pg[:],
                                     xt2_t[p][:, 128 * m:128 * (m + 1)],
                                     xt1_t[p][:, BW * blk:BW * (blk + 1)],
                                     start=(p == 0), stop=(p == 6))
                nc.scalar.copy(gt[:, m, :], pg[:])

            if blk >= 1:
                emit_4a(blk - 1)
            for sub in range(2):
                t = 2 * blk + sub
                cs = 128 * sub
                araw = arawp.tile([128, D], DT.bfloat16, tag="araw")
                for k in range(NCHUNK):
                    span = int(spans[k])
                    if span == 0:
                        continue
                    mt, r0 = k // 2, (k % 2) * 64
                    k2 = k + 1
                    mult = int(n_straddle[k2]) if k2 < NCHUNK else 0
                    npieces = (span + 511) // 512
                    for pc in range(npieces):
                        pw = min(512, span - 512 * pc)
                        last = pc == npieces - 1
                        pt = ps3.tile([128, 512], DT.float32, tag="ps3")
                        nc.tensor.matmul(
                            pt[:, :pw],
                            gt[r0:r0 + 64, mt, cs:cs + 128],
                            sheetm_s[r0:r0 + 64, k // 2, 512 * pc:512 * pc + pw],
                            start=True, stop=not (last and mult > 0))
                        if last and mult > 0:
                            smt, sr0 = k2 // 2, (k2 % 2) * 64
                            nc.tensor.matmul(
                                pt[:, pw - 1:pw],
                                gt[sr0:sr0 + mult, smt, cs:cs + 128],
                                svals_s[sr0:sr0 + mult, smt:smt + 1],
                                start=False, stop=True)
                        eng = nc.vector if (k % 2 == 0) else nc.scalar
                        if eng is nc.vector:
                            eng.tensor_copy(
                                araw[:, int(starts[k]) + 512 * pc:
                                     int(starts[k]) + 512 * pc + pw],
                                pt[:, :pw])
                        else:
                            eng.copy(
                                araw[:, int(starts[k]) + 512 * pc:
                                     int(starts[k]) + 512 * pc + pw],
                                pt[:, :pw])
                # ---- fine roll via SBUF->SBUF DMA
                af = afp.tile([128, D], DT.bfloat16, tag="af")
                for (p0, n, r) in groups[t]:
                    if r == 0:
                        nc.sync.dma_start(af[p0:p0 + n, :], araw[p0:p0 + n, :])
                    else:
                        nc.sync.dma_start(af[p0:p0 + n, r:],
                                          araw[p0:p0 + n, :D - r])
                        nc.sync.dma_start(af[p0:p0 + n, :r],
                                          araw[p0:p0 + n, D - r:])
                af_tiles[t] = af

        emit_4a(NBLK - 1)

        # ---- classifier weights (reuses the sheet slot; sheet is dead now)
        wsb = bigp.tile([128, 64 * NCLS], DT.bfloat16, tag="sheet")
        nc.gpsimd.dma_start(wsb[:], wcls_d[:])

        # ---- 4b: 64 static rotations
        qbf = arawp.tile([128, D], DT.bfloat16, tag="araw")
        nc.vector.tensor_copy(qbf[:64, :D // 2], q_s[:, :D // 2])
        nc.vector.tensor_copy(qbf[:64, D // 2:], q_s[:, D // 2:])
        y2 = psz.tile([64, 128], DT.float32, tag="y2")
        for v in range(NQ):
            nc.tensor.matmul(y2[:], di64_s[:, 64 - v:128 - v],
                             qbf[:64, 128 * v:128 * (v + 1)],
                             start=(v == 0), stop=(v == NQ - 1))

        # ---- post: feat = sign(y)*sqrt(|y|) / max(||.||, eps)
        y2s = postp.tile([64, 128], DT.float32, tag="y2s")
        nc.vector.tensor_copy(y2s[:], y2[:])
        absy = postp.tile([64, 128], DT.float32, tag="absy")
        nc.vector.scalar_tensor_tensor(absy[:], y2s[:], -1.0, y2s[:],
                                       op0=mybir.AluOpType.mult,
                                       op1=mybir.AluOpType.max)
        mag = postp.tile([64, 128], DT.float32, tag="mag")
        nc.scalar.sqrt(mag[:], absy[:])
        nrm2 = postp.tile([64, 1], DT.float32, tag="nrm2")
        nc.vector.reduce_sum(nrm2[:], absy[:], axis=mybir.AxisListType.X)
        npsum = psz.tile([1, 1], DT.float32, tag="pz")
        nc.tensor.matmul(npsum[:], ones64_s[:], nrm2[:], start=True, stop=True)
        nroot = postp.tile([1, 1], DT.float32, tag="nroot")
        nc.scalar.sqrt(nroot[:], npsum[:])
        nclamp = postp.tile([1, 1], DT.float32, tag="nclamp")
        nc.vector.tensor_scalar(nclamp[:], nroot[:], 1e-12, None,
                                op0=mybir.AluOpType.max)
        rn = postp.tile([1, 1], DT.float32, tag="rn")
        nc.vector.reciprocal(rn[:], nclamp[:])
        rnps = psz.tile([64, 1], DT.float32, tag="pz")
        nc.tensor.matmul(rnps[:], ones164_s[:], rn[:], start=True, stop=True)
        rn64 = postp.tile([64, 1], DT.float32, tag="rn64")
        nc.vector.tensor_copy(rn64[:], rnps[:])

        # featn = (2*(y>=0)*mag - mag) * rn  == sign(y)*mag*rn
        gpos = postp.tile([64, 128], DT.float32, tag="gpos")
        nc.vector.scalar_tensor_tensor(gpos[:], y2s[:], 0.0, mag[:],
                                       op0=mybir.AluOpType.is_ge,
                                       op1=mybir.AluOpType.mult)
        feats = postp.tile([64, 128], DT.float32, tag="feats")
        nc.vector.scalar_tensor_tensor(feats[:], gpos[:], 2.0, mag[:],
                                       op0=mybir.AluOpType.mult,
                                       op1=mybir.AluOpType.subtract)
        featn = postp.tile([64, 128], DT.float32, tag="featn")
        nc.vector.tensor_scalar(featn[:], feats[:], rn64[:], None,
                                op0=mybir.AluOpType.mult)
        nc.sync.dma_start(feat_out[:], featn[:])

        featbf = postp.tile([64, 128], DT.bfloat16, tag="featbf")
        nc.vector.tensor_copy(featbf[:], featn[:])
        # transpose feat to [128 e, 64 w] for the classifier contraction
        ftps = psz.tile([128, 64], DT.bfloat16, tag="pz")
        nc.tensor.transpose(ftps[:], featbf[:], di64_s[:, :64])
        featT = postp.tile([128, 64], DT.bfloat16, tag="featT")
        nc.vector.tensor_copy(featT[:], ftps[:])

        # ---- logits
        lps = psz.tile([1, NCLS], DT.float32, tag="pz")
        wv = wsb[:].rearrange("p (w n) -> p w n", w=64)
        for w in range(64):
            nc.tensor.matmul(lps[:], featT[:, w:w + 1], wv[:, w, :],
                             start=(w == 0), stop=(w == 63))
        logit_s = postp.tile([1, NCLS], DT.float32, tag="logit")
        nc.vector.tensor_tensor(logit_s[:], lps[:], bcls_s[:],
                                op=mybir.AluOpType.add)
        nc.sync.dma_start(logit_out[:], logit_s[:])

    nc.compile()
    return nc


def _get_compiled(h1, h2):
    key = (np.asarray(h1).tobytes(), np.asarray(h2).tobytes())
    if key not in _CACHE:
        meta = _host_prep(h1, h2)
        nc = _build(meta)
        _CACHE[key] = (nc, meta)
    return _CACHE[key]


def _make_in_maps(x, s1, s2, W_cls, b_cls, h1, h2, meta):
    s1 = np.asarray(s1, np.float32)
    s2 = np.asarray(s2, np.float32)
    baked = _bake_tensors(meta, s1, s2)
    shared = {
        "sheetm": baked["sheetm"],
        "svals": baked["svals"],
        "oq": baked["oq"],
        "di64": baked["di64"],
        "ones64": baked["ones64"],
        "ones164": baked["ones164"],
        "wcls": np.ascontiguousarray(np.asarray(W_cls, np.float32).reshape(64, 128, NCLS).transpose(1, 0, 2)).reshape(128, 64 * NCLS),
        "bcls": np.asarray(b_cls, np.float32).reshape(1, NCLS),
    }
    perm1, perm2 = meta["perm1"], meta["perm2"]
    in_maps = []
    for core in range(NCORES):
        xc = np.asarray(x[core], np.float32).reshape(C, L)
        m = dict(shared)
        m["xt1"] = np.ascontiguousarray(xc[perm1].T).astype(BF16)
        m["xt2"] = np.ascontiguousarray(xc[perm2].T).astype(BF16)
        in_maps.append(m)
    return in_maps


def kernel(x, s1, s2, W_cls, b_cls, h1, h2):
    nc, meta = _get_compiled(h1, h2)
    in_maps = _make_in_maps(x, s1, s2, W_cls, b_cls, h1, h2, meta)
    res = run_bass_kernel_spmd(nc, in_maps, core_ids=list(range(NCORES)))
    logit = np.stack([res.results[i]["logit_out"].reshape(NCLS)
                      for i in range(NCORES)]).astype(np.float32)
    feat = np.stack([res.results[i]["feat_out"].reshape(D)
                     for i in range(NCORES)]).astype(np.float32)
    return logit, feat


# revision 19
# speedup vs baseline: 1.2062x; 1.2062x over previous
"""Trainium2 Bass kernel for Compact Bilinear Pooling (count-sketch + circular conv)
+ signed-sqrt + L2 norm + linear classifier.

Math: y[d] = sum_{c1,c2} Gram[c1,c2] * s1[c1] * s2[c2] * [(h1[c1]+h2[c2]) mod D == d]
(the reference's per-pixel count-sketch FFT circular convolution summed over pixels
factorizes over channel pairs sharing a pixel index).

Device pipeline per core (one batch element, pure data parallel over B=8):
  1. Gram G'T = X2p^T X1p (bf16 matmuls, fp32 PSUM); c2 rows h2-sorted, c1 rows
     sorted by h1 mod 128.
  2. Scatter-by-h2: per 128-rank chunk, one-hot matmuls into the chunk's bin span
     (+rank-1 "straddler" matmuls for hash values crossing chunk boundaries).
  3. Roll-by-h1: fine shift (h1 mod 128) via contiguous equal-shift row-group
     SBUF->SBUF DMAs; coarse shift via group-sum matmul (one-hot over h1 div 128)
     + 64 static rotation matmuls (slices of a double identity).
  4. signed-sqrt, L2 normalize (||feat||^2 == sum|y|), classifier matmul.

h1/h2 are known at kernel-build time; hash-dependent structure is baked into
access patterns / constant input tensors.
"""
import numpy as np
import ml_dtypes
from contextlib import ExitStack

import concourse.bacc as bacc
import concourse.mybir as mybir
import concourse.tile as tile
from concourse.bass_utils import run_bass_kernel_spmd

BF16 = ml_dtypes.bfloat16
B, C, D, L = 8, 2048, 8192, 784
NCLS = 200
NCORES = 8
FINE = 64             # fine shift modulus (h1 % FINE)
NQ = D // FINE        # 128 coarse groups (h1 // FINE)
DX = D + FINE         # extended A width (wrap-free fine rolls)
RCHUNK = 128          # h2-sorted ranks per stage-3 chunk
NCHUNK = C // RCHUNK  # 16
BW = 256              # c1 block width for Gram/stage-3 pipelining
NBLK = C // BW        # 8
NT = C // 128         # 16 c1 tiles
DT = mybir.dt

# evac routing knobs: s3 evac ACT fraction (out of 10), 4a add route pattern
S3_ACT_MOD = 5        # ne % 10 < S3_ACT_MOD -> ACT else DVE
ADD_GPS_MOD = 0       # chunk n % 3 < ADD_GPS_MOD -> gpsimd add (via scratch) else DVE direct

_CACHE = {}


def _host_prep(h1, h2):
    h1 = np.asarray(h1).astype(np.int64)
    h2 = np.asarray(h2).astype(np.int64)
    perm1 = np.argsort(h1 % FINE, kind="stable")
    perm2 = np.argsort(h2, kind="stable")
    h2s = h2[perm2]
    rho = (h1[perm1] % FINE).astype(np.int64)
    qg = (h1[perm1] // FINE).astype(np.int64)

    # value -> chunk of its first rank
    first_rank = {}
    for i in range(C):
        v = int(h2s[i])
        if v not in first_rank:
            first_rank[v] = i
    cofv = {v: fr // RCHUNK for v, fr in first_rank.items()}

    starts = np.zeros(NCHUNK, np.int64)
    ends = np.zeros(NCHUNK, np.int64)
    cur = 0
    for k in range(NCHUNK):
        starts[k] = cur
        vals_k = [v for v, c in cofv.items() if c == k]
        nxt = D if k == NCHUNK - 1 else (max(vals_k) + 1 if vals_k else cur)
        ends[k] = nxt
        cur = nxt
    assert ends[-1] == D
    spans = (ends - starts).astype(np.int64)

    # compact sheet column offsets (pad each chunk's segment to 8 cols)
    offs = np.zeros(NCHUNK, np.int64)
    cur = 0
    for k in range(NCHUNK):
        offs[k] = cur
        cur += ((int(spans[k]) + 7) // 8) * 8
    SW = int(cur)

    sheet_col = np.full(C, -1, np.int64)
    is_straddle = np.zeros(C, bool)
    n_straddle = np.zeros(NCHUNK, np.int64)
    for i in range(C):
        v = int(h2s[i])
        k_own = i // RCHUNK
        k_asg = cofv[v]
        if k_asg == k_own:
            sheet_col[i] = v - starts[k_own]
        else:
            assert k_asg == k_own - 1
            assert v == ends[k_asg] - 1
            is_straddle[i] = True
            n_straddle[k_own] += 1
    for k in range(NCHUNK):
        ns = int(n_straddle[k])
        assert is_straddle[RCHUNK * k:RCHUNK * k + ns].all()
        assert not is_straddle[RCHUNK * k + ns:RCHUNK * (k + 1)].any()
        assert ns <= RCHUNK

    # fine-roll groups per c1 tile: (p0, n, rho)
    groups = []
    for t in range(NT):
        g = []
        seg = rho[128 * t:128 * (t + 1)]
        p0 = 0
        for p in range(1, 129):
            if p == 128 or seg[p] != seg[p0]:
                g.append((p0, p - p0, int(seg[p0])))
                p0 = p
        groups.append(g)

    return dict(perm1=perm1, perm2=perm2, h2s=h2s, rho=rho, qg=qg,
                starts=starts, ends=ends, spans=spans, offs=offs, SW=SW,
                sheet_col=sheet_col, is_straddle=is_straddle,
                n_straddle=n_straddle, groups=groups)


def _bake_tensors(meta, s1, s2):
    perm1, perm2 = meta["perm1"], meta["perm2"]
    SW, offs = meta["SW"], meta["offs"]
    sheet_col, is_straddle = meta["sheet_col"], meta["is_straddle"]
    qg = meta["qg"]

    # compact sheet [128, SW]: rank i = 128*k + p -> [p, offs[k] + col]
    sheetm = np.zeros((128, SW), np.float32)
    svals = np.zeros((128, NCHUNK), np.float32)
    for i in range(C):
        p, k = i % 128, i // 128
        if is_straddle[i]:
            svals[p, k] = s2[perm2[i]]
        else:
            sheetm[p, offs[k] + sheet_col[i]] = s2[perm2[i]]

    oq = np.zeros((128, NT, NQ), np.float32)
    for j in range(C):
        r, t = j % 128, j // 128
        oq[r, t, qg[j]] = s1[perm1[j]]

    di = np.zeros((128, 256), np.float32)
    for m in range(128):
        di[m, m] = 1.0
        di[m, m + 128] = 1.0

    return dict(
        sheetm=sheetm.astype(BF16),
        svals=svals.astype(BF16),
        oq=oq.astype(BF16),
        di=di.astype(BF16),
        ones128=np.ones((128, 1), np.float32),
        ones1x=np.ones((1, 128), np.float32),
    )


def _build(meta):
    nc = bacc.Bacc("TRN2", target_bir_lowering=False, debug=False,
                   num_devices=NCORES)
    SW = meta["SW"]
    starts, spans, offs = meta["starts"], meta["spans"], meta["offs"]
    n_straddle = meta["n_straddle"]
    groups = meta["groups"]

    def din(name, shape, dt):
        return nc.dram_tensor(name, shape, dt, kind="ExternalInput").ap()

    xt1 = din("xt1", [L, C], DT.bfloat16)
    xt2 = din("xt2", [L, C], DT.bfloat16)
    sheetm_d = din("sheetm", [128, SW], DT.bfloat16)
    svals_d = din("svals", [128, NCHUNK], DT.bfloat16)
    oq_d = din("oq", [128, NT, NQ], DT.bfloat16)
    di_d = din("di", [128, 256], DT.bfloat16)
    ones128_d = din("ones128", [128, 1], DT.float32)
    ones1x_d = din("ones1x", [1, 128], DT.float32)
    wcls_d = din("wcls", [128, 64 * NCLS], DT.float32)
    bcls_d = din("bcls", [1, NCLS], DT.float32)
    feat_out = nc.dram_tensor("feat_out", [128, 64], DT.float32,
                              kind="ExternalOutput").ap()
    logit_out = nc.dram_tensor("logit_out", [1, NCLS], DT.float32,
                               kind="ExternalOutput").ap()

    with tile.TileContext(nc) as tc, ExitStack() as ctx:
        consts = ctx.enter_context(tc.tile_pool(name="consts", bufs=1))
        xtp = ctx.enter_context(tc.tile_pool(name="xtp", bufs=1))
        gtp = ctx.enter_context(tc.tile_pool(name="gtp", bufs=2))
        apool = ctx.enter_context(tc.tile_pool(name="apool", bufs=3))
        qp = ctx.enter_context(tc.tile_pool(name="qp", bufs=1))
        postp = ctx.enter_context(tc.tile_pool(name="postp", bufs=1))
        psg = ctx.enter_context(tc.tile_pool(name="psg", bufs=3, space="PSUM"))
        ps3 = ctx.enter_context(tc.tile_pool(name="ps3", bufs=3, space="PSUM"))
        ps4 = ctx.enter_context(tc.tile_pool(name="ps4", bufs=2, space="PSUM"))

        # ---- constant + weight loads (W early: SWDGE, overlaps everything)
        sheetm_s = consts.tile([128, SW], DT.bfloat16, tag="sheet")
        nc.sync.dma_start(sheetm_s[:], sheetm_d[:])
        svals_s = consts.tile([128, NCHUNK], DT.bfloat16, tag="svals")
        nc.sync.dma_start(svals_s[:], svals_d[:])
        oq_s = consts.tile([128, NT, NQ], DT.bfloat16, tag="oq")
        nc.sync.dma_start(oq_s[:], oq_d[:])
        di_s = consts.tile([128, 256], DT.bfloat16, tag="di")
        nc.sync.dma_start(di_s[:], di_d[:])
        ones128_s = consts.tile([128, 1], DT.float32, tag="ones128")
        nc.sync.dma_start(ones128_s[:], ones128_d[:])
        ones1x_s = consts.tile([1, 128], DT.float32, tag="ones1x")
        nc.sync.dma_start(ones1x_s[:], ones1x_d[:])
        bcls_s = consts.tile([1, NCLS], DT.float32, tag="bcls")
        nc.sync.dma_start(bcls_s[:], bcls_d[:])
        wsb = consts.tile([128, 64 * NCLS], DT.bfloat16, tag="wsb")
        nc.gpsimd.dma_start(wsb[:], wcls_d[:])

        # ---- X^T loads (7 L-partition tiles per side)
        xt1_t, xt2_t = [], []
        for p in range(7):
            rows = 128 if p < 6 else L - 768
            t1 = xtp.tile([rows, C], DT.bfloat16, tag=f"xt1_{p}")
            nc.sync.dma_start(t1[:], xt1[128 * p:128 * p + rows, :])
            xt1_t.append(t1)
            t2 = xtp.tile([rows, C], DT.bfloat16, tag=f"xt2_{p}")
            nc.sync.dma_start(t2[:], xt2[128 * p:128 * p + rows, :])
            xt2_t.append(t2)

        q_s = qp.tile([128, DX], DT.float32, tag="q")
        scrp = ctx.enter_context(tc.tile_pool(name="scrp", bufs=2))
        af_tiles = [None] * NT

        def emit_4a(j):
            for n in range(17):
                c0 = 512 * n
                cw = 512 if n < 16 else FINE
                pt = ps4.tile([128, 512], DT.float32, tag="ps4")
                nc.tensor.matmul(pt[:, :cw], oq_s[:, 2 * j, :],
                                 af_tiles[2 * j][:, c0:c0 + cw],
                                 start=True, stop=False)
                nc.tensor.matmul(pt[:, :cw], oq_s[:, 2 * j + 1, :],
                                 af_tiles[2 * j + 1][:, c0:c0 + cw],
                                 start=False, stop=True)
                qsl = q_s[:, c0:c0 + cw]
                if j == 0:
                    if n % 2 == 0:
                        nc.vector.tensor_copy(qsl, pt[:, :cw])
                    else:
                        nc.scalar.copy(qsl, pt[:, :cw])
                elif n % 3 < ADD_GPS_MOD:
                    scr = scrp.tile([128, 512], DT.float32, tag="scr")
                    if n % 2 == 0:
                        nc.vector.tensor_copy(scr[:, :cw], pt[:, :cw])
                    else:
                        nc.scalar.copy(scr[:, :cw], pt[:, :cw])
                    nc.gpsimd.tensor_tensor(qsl, qsl, scr[:, :cw],
                                            op=mybir.AluOpType.add)
                else:
                    nc.vector.tensor_tensor(qsl, qsl, pt[:, :cw],
                                            op=mybir.AluOpType.add)
            af_tiles[2 * j] = None
            af_tiles[2 * j + 1] = None

        def emit_s3(t, gt):
            sub = t % 2
            cs = 128 * sub
            araw = apool.tile([128, DX], DT.bfloat16, tag="ab", name=f"araw{t}")
            ne = 0
            for k in range(NCHUNK):
                span = int(spans[k])
                if span == 0:
                    continue
                k2 = k + 1
                mult = int(n_straddle[k2]) if k2 < NCHUNK else 0
                npieces = (span + 511) // 512
                for pc in range(npieces):
                    pw = min(512, span - 512 * pc)
                    last = pc == npieces - 1
                    pt = ps3.tile([128, 512], DT.float32, tag="ps3")
                    nc.tensor.matmul(
                        pt[:, :pw],
                        gt[:, k, cs:cs + 128],
                        sheetm_s[:, int(offs[k]) + 512 * pc:
                                 int(offs[k]) + 512 * pc + pw],
                        start=True, stop=not (last and mult > 0))
                    if last and mult > 0:
                        nc.tensor.matmul(
                            pt[:, pw - 1:pw],
                            gt[0:mult, k2, cs:cs + 128],
                            svals_s[0:mult, k2:k2 + 1],
                            start=False, stop=True)
                    dst = araw[:, int(starts[k]) + 512 * pc:
                               int(starts[k]) + 512 * pc + pw]
                    if ne % 3 == 0:
                        nc.vector.tensor_copy(dst, pt[:, :pw])
                    else:
                        nc.scalar.copy(dst, pt[:, :pw])
                    ne += 1
            # fine roll via wrap-free SBUF->SBUF DMA (one per group); the
            # first/last FINE columns are zeroed and the wrapped tail is
            # folded back into Q after 4a (apron columns [D, D+FINE)).
            af = apool.tile([128, DX], DT.bfloat16, tag="ab", name=f"af{t}")
            nc.gpsimd.memset(af[:, 0:FINE], 0.0)
            nc.gpsimd.memset(af[:, D:DX], 0.0)
            for gi, (p0, n, r) in enumerate(groups[t]):
                eng = nc.sync if gi % 2 == 0 else nc.scalar
                eng.dma_start(af[p0:p0 + n, r:r + D], araw[p0:p0 + n, 0:D])
            af_tiles[t] = af

        for blk in range(NBLK):
            # ---- Gram block: G'T[:, blk*BW : (blk+1)*BW]
            gt = gtp.tile([128, NT, BW], DT.bfloat16, tag="gt")
            for m in range(NT):
                pg = psg.tile([128, BW], DT.float32, tag="psg")
                for p in range(7):
                    nc.tensor.matmul(pg[:],
                                     xt2_t[p][:, 128 * m:128 * (m + 1)],
                                     xt1_t[p][:, BW * blk:BW * (blk + 1)],
                                     start=(p == 0), stop=(p == 6))
                if m % 2 == 0:
                    nc.scalar.copy(gt[:, m, :], pg[:])
                else:
                    nc.vector.tensor_copy(gt[:, m, :], pg[:])
            emit_s3(2 * blk, gt)
            if blk >= 1:
                emit_4a(blk - 1)
            emit_s3(2 * blk + 1, gt)

        emit_4a(NBLK - 1)

        # fold the wrap apron back: Q[:, 0:FINE] += Q[:, D:DX]
        nc.vector.tensor_tensor(q_s[:, 0:FINE], q_s[:, 0:FINE],
                                q_s[:, D:DX], op=mybir.AluOpType.add)

        # ---- 4b: 128 static rotations (roll Q[m] right by FINE*m, sum)
        qbf = apool.tile([128, DX], DT.bfloat16, tag="ab", name="qbf")
        nc.vector.tensor_copy(qbf[:, :D // 2], q_s[:, :D // 2])
        nc.scalar.copy(qbf[:, D // 2:D], q_s[:, D // 2:D])
        y2 = ps4.tile([128, 64], DT.float32, tag="ps4")
        for v in range(NQ):
            nc.tensor.matmul(y2[:], di_s[:, 128 - v:256 - v],
                             qbf[:, FINE * v:FINE * (v + 1)],
                             start=(v == 0), stop=(v == NQ - 1))

        # ---- post: feat = sign(y)*sqrt(|y|) / max(||.||, eps)
        y2s = postp.tile([128, 64], DT.float32, tag="y2s")
        nc.vector.tensor_copy(y2s[:], y2[:])
        absy = postp.tile([128, 64], DT.float32, tag="absy")
        nc.vector.scalar_tensor_tensor(absy[:], y2s[:], -1.0, y2s[:],
                                       op0=mybir.AluOpType.mult,
                                       op1=mybir.AluOpType.max)
        mag = postp.tile([128, 64], DT.float32, tag="mag")
        nc.scalar.sqrt(mag[:], absy[:])
        nrm2 = postp.tile([128, 1], DT.float32, tag="nrm2")
        nc.vector.reduce_sum(nrm2[:], absy[:], axis=mybir.AxisListType.X)
        npsum = ps4.tile([1, 1], DT.float32, tag="ps4")
        nc.tensor.matmul(npsum[:], ones128_s[:], nrm2[:], start=True, stop=True)
        nroot = postp.tile([1, 1], DT.float32, tag="nroot")
        nc.scalar.sqrt(nroot[:], npsum[:])
        nclamp = postp.tile([1, 1], DT.float32, tag="nclamp")
        nc.vector.tensor_scalar(nclamp[:], nroot[:], 1e-12, None,
                                op0=mybir.AluOpType.max)
        rn = postp.tile([1, 1], DT.float32, tag="rn")
        nc.vector.reciprocal(rn[:], nclamp[:])
        rnps = ps4.tile([128, 1], DT.float32, tag="ps4")
        nc.tensor.matmul(rnps[:], ones1x_s[:], rn[:], start=True, stop=True)
        rn128 = postp.tile([128, 1], DT.float32, tag="rn128")
        nc.vector.tensor_copy(rn128[:], rnps[:])

        # featn = (2*(y>=0)*mag - mag) * rn  == sign(y)*mag*rn
        gpos = postp.tile([128, 64], DT.float32, tag="gpos")
        nc.vector.scalar_tensor_tensor(gpos[:], y2s[:], 0.0, mag[:],
                                       op0=mybir.AluOpType.is_ge,
                                       op1=mybir.AluOpType.mult)
        feats = postp.tile([128, 64], DT.float32, tag="feats")
        nc.vector.scalar_tensor_tensor(feats[:], gpos[:], 2.0, mag[:],
                                       op0=mybir.AluOpType.mult,
                                       op1=mybir.AluOpType.subtract)
        featn = postp.tile([128, 64], DT.float32, tag="featn")
        nc.vector.tensor_scalar(featn[:], feats[:], rn128[:], None,
                                op0=mybir.AluOpType.mult)
        nc.sync.dma_start(feat_out[:], featn[:])

        featbf = postp.tile([128, 64], DT.bfloat16, tag="featbf")
        nc.vector.tensor_copy(featbf[:], featn[:])

        # ---- logits: contract d = 64u + e over u (partitions) per e
        lps = ps4.tile([1, NCLS], DT.float32, tag="ps4")
        wv = wsb[:].rearrange("p (e n) -> p e n", e=64)
        for e in range(64):
            nc.tensor.matmul(lps[:], featbf[:, e:e + 1], wv[:, e, :],
                             start=(e == 0), stop=(e == 63))
        logit_s = postp.tile([1, NCLS], DT.float32, tag="logit")
        nc.vector.tensor_tensor(logit_s[:], lps[:], bcls_s[:],
                                op=mybir.AluOpType.add)
        nc.sync.dma_start(logit_out[:], logit_s[:])

    nc.compile()
    return nc


def _get_compiled(h1, h2):
    key = (np.asarray(h1).tobytes(), np.asarray(h2).tobytes())
    if key not in _CACHE:
        meta = _host_prep(h1, h2)
        nc = _build(meta)
        _CACHE[key] = (nc, meta)
    return _CACHE[key]


def _make_in_maps(x, s1, s2, W_cls, b_cls, h1, h2, meta):
    s1 = np.asarray(s1, np.float32)
    s2 = np.asarray(s2, np.float32)
    baked = _bake_tensors(meta, s1, s2)
    shared = {
        "sheetm": baked["sheetm"],
        "svals": baked["svals"],
        "oq": baked["oq"],
        "di": baked["di"],
        "ones128": baked["ones128"],
        "ones1x": baked["ones1x"],
        "wcls": np.ascontiguousarray(np.asarray(W_cls, np.float32)).reshape(128, 64 * NCLS),
        "bcls": np.asarray(b_cls, np.float32).reshape(1, NCLS),
    }
    perm1, perm2 = meta["perm1"], meta["perm2"]
    in_maps = []
    for core in range(NCORES):
        xc = np.asarray(x[core], np.float32).reshape(C, L)
        m = dict(shared)
        m["xt1"] = np.ascontiguousarray(xc[perm1].T).astype(BF16)
        m["xt2"] = np.ascontiguousarray(xc[perm2].T).astype(BF16)
        in_maps.append(m)
    return in_maps


def kernel(x, s1, s2, W_cls, b_cls, h1, h2):
    nc, meta = _get_compiled(h1, h2)
    in_maps = _make_in_maps(x, s1, s2, W_cls, b_cls, h1, h2, meta)
    res = run_bass_kernel_spmd(nc, in_maps, core_ids=list(range(NCORES)))
    logit = np.stack([res.results[i]["logit_out"].reshape(NCLS)
                      for i in range(NCORES)]).astype(np.float32)
    feat = np.stack([res.results[i]["feat_out"].reshape(D)
                     for i in range(NCORES)]).astype(np.float32)
    return logit, feat


# revision 21
# speedup vs baseline: 1.3032x; 1.0805x over previous
"""Trainium2 Bass kernel for Compact Bilinear Pooling (count-sketch + circular conv)
+ signed-sqrt + L2 norm + linear classifier.

Math: y[d] = sum_{c1,c2} Gram[c1,c2] * s1[c1] * s2[c2] * [(h1[c1]+h2[c2]) mod D == d]
(the reference's per-pixel count-sketch FFT circular convolution summed over pixels
factorizes over channel pairs sharing a pixel index).

Device pipeline per core (one batch element, pure data parallel over B=8):
  1. Gram G'T = X2p^T X1p (bf16 matmuls, fp32 PSUM); c2 rows h2-sorted, c1 rows
     sorted by h1 mod 128.
  2. Scatter-by-h2: per 128-rank chunk, one-hot matmuls into the chunk's bin span
     (+rank-1 "straddler" matmuls for hash values crossing chunk boundaries).
  3. Roll-by-h1: fine shift (h1 mod 128) via contiguous equal-shift row-group
     SBUF->SBUF DMAs; coarse shift via group-sum matmul (one-hot over h1 div 128)
     + 64 static rotation matmuls (slices of a double identity).
  4. signed-sqrt, L2 normalize (||feat||^2 == sum|y|), classifier matmul.

h1/h2 are known at kernel-build time; hash-dependent structure is baked into
access patterns / constant input tensors.
"""
import numpy as np
import ml_dtypes
from contextlib import ExitStack

import concourse.bacc as bacc
import concourse.mybir as mybir
import concourse.tile as tile
from concourse.bass_utils import run_bass_kernel_spmd

BF16 = ml_dtypes.bfloat16
B, C, D, L = 8, 2048, 8192, 784
NCLS = 200
NCORES = 8
FINE = 64             # fine shift modulus (h1 % FINE)
NQ = D // FINE        # 128 coarse groups (h1 // FINE)
DX = D + FINE         # extended A width (wrap-free fine rolls)
RCHUNK = 128          # h2-sorted ranks per stage-3 chunk
NCHUNK = C // RCHUNK  # 16
BW = 256              # c1 block width for Gram/stage-3 pipelining
NBLK = C // BW        # 8
NT = C // 128         # 16 c1 tiles
DT = mybir.dt

# evac routing knobs: s3 evac ACT fraction (out of 10), 4a add route pattern
S3_ACT_MOD = 5        # ne % 10 < S3_ACT_MOD -> ACT else DVE
ADD_GPS_MOD = 0       # chunk n % 3 < ADD_GPS_MOD -> gpsimd add (via scratch) else DVE direct

_CACHE = {}


def _host_prep(h1, h2):
    h1 = np.asarray(h1).astype(np.int64)
    h2 = np.asarray(h2).astype(np.int64)
    perm1 = np.argsort(h1 % FINE, kind="stable")
    perm2 = np.argsort(h2, kind="stable")
    h2s = h2[perm2]
    rho = (h1[perm1] % FINE).astype(np.int64)
    qg = (h1[perm1] // FINE).astype(np.int64)

    # value -> chunk of its first rank
    first_rank = {}
    for i in range(C):
        v = int(h2s[i])
        if v not in first_rank:
            first_rank[v] = i
    cofv = {v: fr // RCHUNK for v, fr in first_rank.items()}

    starts = np.zeros(NCHUNK, np.int64)
    ends = np.zeros(NCHUNK, np.int64)
    cur = 0
    for k in range(NCHUNK):
        starts[k] = cur
        vals_k = [v for v, c in cofv.items() if c == k]
        nxt = D if k == NCHUNK - 1 else (max(vals_k) + 1 if vals_k else cur)
        ends[k] = nxt
        cur = nxt
    assert ends[-1] == D
    spans = (ends - starts).astype(np.int64)

    # compact sheet column offsets (pad each chunk's segment to 8 cols)
    offs = np.zeros(NCHUNK, np.int64)
    cur = 0
    for k in range(NCHUNK):
        offs[k] = cur
        cur += ((int(spans[k]) + 7) // 8) * 8
    SW = int(cur)

    sheet_col = np.full(C, -1, np.int64)
    is_straddle = np.zeros(C, bool)
    n_straddle = np.zeros(NCHUNK, np.int64)
    for i in range(C):
        v = int(h2s[i])
        k_own = i // RCHUNK
        k_asg = cofv[v]
        if k_asg == k_own:
            sheet_col[i] = v - starts[k_own]
        else:
            assert k_asg == k_own - 1
            assert v == ends[k_asg] - 1
            is_straddle[i] = True
            n_straddle[k_own] += 1
    for k in range(NCHUNK):
        ns = int(n_straddle[k])
        assert is_straddle[RCHUNK * k:RCHUNK * k + ns].all()
        assert not is_straddle[RCHUNK * k + ns:RCHUNK * (k + 1)].any()
        assert ns <= RCHUNK

    # fine-roll groups per c1 tile: (p0, n, rho)
    groups = []
    for t in range(NT):
        g = []
        seg = rho[128 * t:128 * (t + 1)]
        p0 = 0
        for p in range(1, 129):
            if p == 128 or seg[p] != seg[p0]:
                g.append((p0, p - p0, int(seg[p0])))
                p0 = p
        groups.append(g)

    return dict(perm1=perm1, perm2=perm2, h2s=h2s, rho=rho, qg=qg,
                starts=starts, ends=ends, spans=spans, offs=offs, SW=SW,
                sheet_col=sheet_col, is_straddle=is_straddle,
                n_straddle=n_straddle, groups=groups)


def _bake_tensors(meta, s1, s2):
    perm1, perm2 = meta["perm1"], meta["perm2"]
    SW, offs = meta["SW"], meta["offs"]
    sheet_col, is_straddle = meta["sheet_col"], meta["is_straddle"]
    qg = meta["qg"]

    # compact sheet [128, SW]: rank i = 128*k + p -> [p, offs[k] + col]
    sheetm = np.zeros((128, SW), np.float32)
    svals = np.zeros((128, NCHUNK), np.float32)
    for i in range(C):
        p, k = i % 128, i // 128
        if is_straddle[i]:
            svals[p, k] = s2[perm2[i]]
        else:
            sheetm[p, offs[k] + sheet_col[i]] = s2[perm2[i]]

    oq = np.zeros((128, NT, NQ), np.float32)
    for j in range(C):
        r, t = j % 128, j // 128
        oq[r, t, qg[j]] = s1[perm1[j]]

    di = np.zeros((128, 256), np.float32)
    for m in range(128):
        di[m, m] = 1.0
        di[m, m + 128] = 1.0

    return dict(
        sheetm=sheetm.astype(BF16),
        svals=svals.astype(BF16),
        oq=oq.astype(BF16),
        di=di.astype(BF16),
        ones128=np.ones((128, 1), np.float32),
        ones1x=np.ones((1, 128), np.float32),
    )


def _build(meta):
    nc = bacc.Bacc("TRN2", target_bir_lowering=False, debug=False,
                   num_devices=NCORES)
    SW = meta["SW"]
    starts, spans, offs = meta["starts"], meta["spans"], meta["offs"]
    n_straddle = meta["n_straddle"]
    groups = meta["groups"]

    def din(name, shape, dt):
        return nc.dram_tensor(name, shape, dt, kind="ExternalInput").ap()

    xt1 = din("xt1", [L, C], DT.bfloat16)
    xt2 = din("xt2", [L, C], DT.bfloat16)
    sheetm_d = din("sheetm", [128, SW], DT.bfloat16)
    svals_d = din("svals", [128, NCHUNK], DT.bfloat16)
    oq_d = din("oq", [128, NT, NQ], DT.bfloat16)
    di_d = din("di", [128, 256], DT.bfloat16)
    ones128_d = din("ones128", [128, 1], DT.float32)
    ones1x_d = din("ones1x", [1, 128], DT.float32)
    wcls_d = din("wcls", [128, 64 * NCLS], DT.float32)
    bcls_d = din("bcls", [1, NCLS], DT.float32)
    feat_out = nc.dram_tensor("feat_out", [128, 64], DT.float32,
                              kind="ExternalOutput").ap()
    logit_out = nc.dram_tensor("logit_out", [1, NCLS], DT.float32,
                               kind="ExternalOutput").ap()

    with tile.TileContext(nc) as tc, ExitStack() as ctx:
        consts = ctx.enter_context(tc.tile_pool(name="consts", bufs=1))
        xtp = ctx.enter_context(tc.tile_pool(name="xtp", bufs=1))
        gtp = ctx.enter_context(tc.tile_pool(name="gtp", bufs=2))
        apool = ctx.enter_context(tc.tile_pool(name="apool", bufs=4))
        qp = ctx.enter_context(tc.tile_pool(name="qp", bufs=1))
        postp = ctx.enter_context(tc.tile_pool(name="postp", bufs=1))
        psg = ctx.enter_context(tc.tile_pool(name="psg", bufs=3, space="PSUM"))
        ps3 = ctx.enter_context(tc.tile_pool(name="ps3", bufs=3, space="PSUM"))
        ps4 = ctx.enter_context(tc.tile_pool(name="ps4", bufs=2, space="PSUM"))

        # ---- constant + weight loads (W early: SWDGE, overlaps everything)
        sheetm_s = consts.tile([128, SW], DT.bfloat16, tag="sheet")
        nc.sync.dma_start(sheetm_s[:], sheetm_d[:])
        svals_s = consts.tile([128, NCHUNK], DT.bfloat16, tag="svals")
        nc.sync.dma_start(svals_s[:], svals_d[:])
        oq_s = consts.tile([128, NT, NQ], DT.bfloat16, tag="oq")
        nc.sync.dma_start(oq_s[:], oq_d[:])
        di_s = consts.tile([128, 256], DT.bfloat16, tag="di")
        nc.sync.dma_start(di_s[:], di_d[:])
        ones128_s = consts.tile([128, 1], DT.float32, tag="ones128")
        nc.sync.dma_start(ones128_s[:], ones128_d[:])
        ones1x_s = consts.tile([1, 128], DT.float32, tag="ones1x")
        nc.sync.dma_start(ones1x_s[:], ones1x_d[:])
        bcls_s = consts.tile([1, NCLS], DT.float32, tag="bcls")
        nc.sync.dma_start(bcls_s[:], bcls_d[:])
        wsb = consts.tile([128, 64 * NCLS], DT.bfloat16, tag="wsb")
        nc.gpsimd.dma_start(wsb[:], wcls_d[:])

        # ---- X^T loads: xt2 resident, xt1 streamed per c1-block
        xt2_t = []
        for p in range(7):
            rows = 128 if p < 6 else L - 768
            t2 = xtp.tile([rows, C], DT.bfloat16, tag=f"xt2_{p}")
            nc.sync.dma_start(t2[:], xt2[128 * p:128 * p + rows, :])
            xt2_t.append(t2)

        def load_xt1_block(blk):
            tiles = []
            for p in range(7):
                rows = 128 if p < 6 else L - 768
                t1 = xtp.tile([rows, BW], DT.bfloat16, tag=f"xt1b_{p}",
                              bufs=2, name=f"xt1b{p}_{blk}")
                nc.sync.dma_start(
                    t1[:], xt1[128 * p:128 * p + rows,
                               BW * blk:BW * (blk + 1)])
                tiles.append(t1)
            return tiles

        q_s = qp.tile([128, DX], DT.float32, tag="q")
        af_tiles = [None] * NT

        def emit_4a(j):
            tiles = [t for t in (3 * j, 3 * j + 1, 3 * j + 2)
                     if t < NT and af_tiles[t] is not None]
            for n in range(17):
                c0 = 512 * n
                cw = 512 if n < 16 else FINE
                pt = ps4.tile([128, 512], DT.float32, tag="ps4")
                for ti, t in enumerate(tiles):
                    nc.tensor.matmul(pt[:, :cw], oq_s[:, t, :],
                                     af_tiles[t][:, c0:c0 + cw],
                                     start=(ti == 0),
                                     stop=(ti == len(tiles) - 1))
                qsl = q_s[:, c0:c0 + cw]
                if j == 0:
                    if n % 2 == 0:
                        nc.vector.tensor_copy(qsl, pt[:, :cw])
                    else:
                        nc.scalar.copy(qsl, pt[:, :cw])
                else:
                    nc.vector.tensor_tensor(qsl, qsl, pt[:, :cw],
                                            op=mybir.AluOpType.add)
            for t in tiles:
                af_tiles[t] = None

        def emit_s3(t, gt):
            sub = t % 2
            cs = 128 * sub
            araw = apool.tile([128, DX], DT.bfloat16, tag="ab", name=f"araw{t}")
            ne = 0
            for k in range(NCHUNK):
                span = int(spans[k])
                if span == 0:
                    continue
                k2 = k + 1
                mult = int(n_straddle[k2]) if k2 < NCHUNK else 0
                npieces = (span + 511) // 512
                for pc in range(npieces):
                    pw = min(512, span - 512 * pc)
                    last = pc == npieces - 1
                    pt = ps3.tile([128, 512], DT.float32, tag="ps3")
                    nc.tensor.matmul(
                        pt[:, :pw],
                        gt[:, k, cs:cs + 128],
                        sheetm_s[:, int(offs[k]) + 512 * pc:
                                 int(offs[k]) + 512 * pc + pw],
                        start=True, stop=not (last and mult > 0))
                    if last and mult > 0:
                        nc.tensor.matmul(
                            pt[:, pw - 1:pw],
                            gt[0:mult, k2, cs:cs + 128],
                            svals_s[0:mult, k2:k2 + 1],
                            start=False, stop=True)
                    dst = araw[:, int(starts[k]) + 512 * pc:
                               int(starts[k]) + 512 * pc + pw]
                    if ne % 3 == 0:
                        nc.vector.tensor_copy(dst, pt[:, :pw])
                    else:
                        nc.scalar.copy(dst, pt[:, :pw])
                    ne += 1
            # fine roll via wrap-free SBUF->SBUF DMA (one per group); the
            # first/last FINE columns are zeroed and the wrapped tail is
            # folded back into Q after 4a (apron columns [D, D+FINE)).
            af = apool.tile([128, DX], DT.bfloat16, tag="ab", name=f"af{t}")
            nc.gpsimd.memset(af[:, 0:FINE], 0.0)
            nc.gpsimd.memset(af[:, D:DX], 0.0)
            for gi, (p0, n, r) in enumerate(groups[t]):
                eng = nc.sync if gi % 2 == 0 else nc.scalar
                eng.dma_start(af[p0:p0 + n, r:r + D], araw[p0:p0 + n, 0:D])
            af_tiles[t] = af

        xt1_blk = load_xt1_block(0)
        for blk in range(NBLK):
            # ---- Gram block: G'T[:, blk*BW : (blk+1)*BW]
            gt = gtp.tile([128, NT, BW], DT.bfloat16, tag="gt")
            for m in range(NT):
                pg = psg.tile([128, BW], DT.float32, tag="psg")
                for p in range(7):
                    nc.tensor.matmul(pg[:],
                                     xt2_t[p][:, 128 * m:128 * (m + 1)],
                                     xt1_blk[p][:],
                                     start=(p == 0), stop=(p == 6))
                if m % 2 == 0:
                    nc.scalar.copy(gt[:, m, :], pg[:])
                else:
                    nc.vector.tensor_copy(gt[:, m, :], pg[:])
            if blk + 1 < NBLK:
                nxt = load_xt1_block(blk + 1)
            emit_s3(2 * blk, gt)
            if (2 * blk) % 3 == 2:
                emit_4a((2 * blk) // 3)
            emit_s3(2 * blk + 1, gt)
            if (2 * blk + 1) % 3 == 2:
                emit_4a((2 * blk + 1) // 3)
            xt1_blk = nxt if blk + 1 < NBLK else None

        emit_4a(5)

        # fold the wrap apron back: Q[:, 0:FINE] += Q[:, D:DX]
        nc.vector.tensor_tensor(q_s[:, 0:FINE], q_s[:, 0:FINE],
                                q_s[:, D:DX], op=mybir.AluOpType.add)

        # ---- 4b: 128 static rotations (roll Q[m] right by FINE*m, sum)
        qbf = apool.tile([128, DX], DT.bfloat16, tag="ab", name="qbf")
        nc.vector.tensor_copy(qbf[:, :D // 2], q_s[:, :D // 2])
        nc.scalar.copy(qbf[:, D // 2:D], q_s[:, D // 2:D])
        y2 = ps4.tile([128, 64], DT.float32, tag="ps4")
        for v in range(NQ):
            nc.tensor.matmul(y2[:], di_s[:, 128 - v:256 - v],
                             qbf[:, FINE * v:FINE * (v + 1)],
                             start=(v == 0), stop=(v == NQ - 1))

        # ---- post: feat = sign(y)*sqrt(|y|) / max(||.||, eps)
        y2s = postp.tile([128, 64], DT.float32, tag="y2s")
        nc.vector.tensor_copy(y2s[:], y2[:])
        absy = postp.tile([128, 64], DT.float32, tag="absy")
        nc.vector.scalar_tensor_tensor(absy[:], y2s[:], -1.0, y2s[:],
                                       op0=mybir.AluOpType.mult,
                                       op1=mybir.AluOpType.max)
        mag = postp.tile([128, 64], DT.float32, tag="mag")
        nc.scalar.sqrt(mag[:], absy[:])
        nrm2 = postp.tile([128, 1], DT.float32, tag="nrm2")
        nc.vector.reduce_sum(nrm2[:], absy[:], axis=mybir.AxisListType.X)
        npsum = ps4.tile([1, 1], DT.float32, tag="ps4")
        nc.tensor.matmul(npsum[:], ones128_s[:], nrm2[:], start=True, stop=True)
        nroot = postp.tile([1, 1], DT.float32, tag="nroot")
        nc.scalar.sqrt(nroot[:], npsum[:])
        nclamp = postp.tile([1, 1], DT.float32, tag="nclamp")
        nc.vector.tensor_scalar(nclamp[:], nroot[:], 1e-12, None,
                                op0=mybir.AluOpType.max)
        rn = postp.tile([1, 1], DT.float32, tag="rn")
        nc.vector.reciprocal(rn[:], nclamp[:])
        rnps = ps4.tile([128, 1], DT.float32, tag="ps4")
        nc.tensor.matmul(rnps[:], ones1x_s[:], rn[:], start=True, stop=True)
        rn128 = postp.tile([128, 1], DT.float32, tag="rn128")
        nc.vector.tensor_copy(rn128[:], rnps[:])

        # featn = (2*(y>=0)*mag - mag) * rn  == sign(y)*mag*rn
        gpos = postp.tile([128, 64], DT.float32, tag="gpos")
        nc.vector.scalar_tensor_tensor(gpos[:], y2s[:], 0.0, mag[:],
                                       op0=mybir.AluOpType.is_ge,
                                       op1=mybir.AluOpType.mult)
        feats = postp.tile([128, 64], DT.float32, tag="feats")
        nc.vector.scalar_tensor_tensor(feats[:], gpos[:], 2.0, mag[:],
                                       op0=mybir.AluOpType.mult,
                                       op1=mybir.AluOpType.subtract)
        featn = postp.tile([128, 64], DT.float32, tag="featn")
        nc.vector.tensor_scalar(featn[:], feats[:], rn128[:], None,
                                op0=mybir.AluOpType.mult)
        nc.sync.dma_start(feat_out[:], featn[:])

        featbf = postp.tile([128, 64], DT.bfloat16, tag="featbf")
        nc.vector.tensor_copy(featbf[:], featn[:])

        # ---- logits: contract d = 64u + e over u (partitions) per e
        lps = ps4.tile([1, NCLS], DT.float32, tag="ps4")
        wv = wsb[:].rearrange("p (e n) -> p e n", e=64)
        for e in range(64):
            nc.tensor.matmul(lps[:], featbf[:, e:e + 1], wv[:, e, :],
                             start=(e == 0), stop=(e == 63))
        logit_s = postp.tile([1, NCLS], DT.float32, tag="logit")
        nc.vector.tensor_tensor(logit_s[:], lps[:], bcls_s[:],
                                op=mybir.AluOpType.add)
        nc.sync.dma_start(logit_out[:], logit_s[:])

    nc.compile()
    return nc


def _get_compiled(h1, h2):
    key = (np.asarray(h1).tobytes(), np.asarray(h2).tobytes())
    if key not in _CACHE:
        meta = _host_prep(h1, h2)
        nc = _build(meta)
        _CACHE[key] = (nc, meta)
    return _CACHE[key]


def _make_in_maps(x, s1, s2, W_cls, b_cls, h1, h2, meta):
    s1 = np.asarray(s1, np.float32)
    s2 = np.asarray(s2, np.float32)
    baked = _bake_tensors(meta, s1, s2)
    shared = {
        "sheetm": baked["sheetm"],
        "svals": baked["svals"],
        "oq": baked["oq"],
        "di": baked["di"],
        "ones128": baked["ones128"],
        "ones1x": baked["ones1x"],
        "wcls": np.ascontiguousarray(np.asarray(W_cls, np.float32)).reshape(128, 64 * NCLS),
        "bcls": np.asarray(b_cls, np.float32).reshape(1, NCLS),
    }
    perm1, perm2 = meta["perm1"], meta["perm2"]
    in_maps = []
    for core in range(NCORES):
        xc = np.asarray(x[core], np.float32).reshape(C, L)
        m = dict(shared)
        m["xt1"] = np.ascontiguousarray(xc[perm1].T).astype(BF16)
        m["xt2"] = np.ascontiguousarray(xc[perm2].T).astype(BF16)
        in_maps.append(m)
    return in_maps


def kernel(x, s1, s2, W_cls, b_cls, h1, h2):
    nc, meta = _get_compiled(h1, h2)
    in_maps = _make_in_maps(x, s1, s2, W_cls, b_cls, h1, h2, meta)
    res = run_bass_kernel_spmd(nc, in_maps, core_ids=list(range(NCORES)))
    logit = np.stack([res.results[i]["logit_out"].reshape(NCLS)
                      for i in range(NCORES)]).astype(np.float32)
    feat = np.stack([res.results[i]["feat_out"].reshape(D)
                     for i in range(NCORES)]).astype(np.float32)
    return logit, feat
